# revision 5
# baseline (speedup 1.0000x reference)
"""LSTM layer kernel for trn2: T=2048, B=32, I=H=512, fp32.

Strategy: data-parallel over batch (B=32 -> 4 per core, 8 cores).
Per core, one bass program with two phases:
  Phase A: input projections xz[t,b,:] = concat_g(x[t,b] @ W_xg.T + b_g)
           as dense GEMMs (weights pre-transposed on chip via PE transpose),
           stored to a DRAM scratch [T, BL, 4H].
  Phase B: the sequential recurrence. State h kept both flat [BL,H] and
           transposed/packed hT [128, 4*BL] so the recurrent matmuls can use
           h.T chunks as the stationary operand:
             z_g[BL,512] = xz_g + sum_k hT[:,k].T @ WhT[k][:, g*512:+512]
           The xz term is injected into PSUM with an identity matmul.
           Gates via ScalarE sigmoid/tanh, c/h updates via VectorE,
           h -> hT via 4 small PE transposes per step.
"""

import numpy as np

T, B, I, H = 2048, 32, 512, 512
NCORES = 8
BL = B // NCORES            # 4 sequences per core
H4 = 4 * H                  # stacked gates i,f,o,c
CH = 16                     # steps per dynamic-loop chunk

GATES = ("i", "f", "o", "c")


def build(t_total=T, ch=CH):
    import concourse.bass as bass
    import concourse.mybir as mybir
    import concourse.tile as tile
    from concourse import bacc
    from concourse.bass import ds, ts
    from concourse.masks import make_identity

    fp32 = mybir.dt.float32
    AF = mybir.ActivationFunctionType
    OP = mybir.AluOpType
    nch = t_total // ch
    KC = I // 128            # 4 contraction chunks

    from contextlib import ExitStack

    nc = bacc.Bacc(None, target_bir_lowering=False)
    names = {}
    with tile.TileContext(nc) as tc:
        with ExitStack() as stack:
            dram = stack.enter_context(
                tc.tile_pool(name="dram", bufs=1, space="DRAM"))
            x_d = dram.tile([t_total, BL, I], fp32, kind="ExternalInput", name="x")
            w_x = {}
            w_h = {}
            b_x = {}
            for g in GATES:
                w_x[g] = dram.tile([H, I], fp32, kind="ExternalInput", name=f"wx{g}")
                w_h[g] = dram.tile([H, H], fp32, kind="ExternalInput", name=f"wh{g}")
                b_x[g] = dram.tile([H], fp32, kind="ExternalInput", name=f"bx{g}")
            y_d = dram.tile([nch, ch, BL, H], fp32, kind="ExternalOutput", name="y")
            hT_d = dram.tile([BL, H], fp32, kind="ExternalOutput", name="hfin")
            cT_d = dram.tile([BL, H], fp32, kind="ExternalOutput", name="cfin")
            xz_d = dram.tile([nch, ch, BL, H4], fp32, kind="Internal", name="xz")

            names["x"] = x_d.name
            for g in GATES:
                names[f"wx{g}"] = w_x[g].name
                names[f"wh{g}"] = w_h[g].name
                names[f"bx{g}"] = b_x[g].name
            names["y"] = y_d.name
            names["hfin"] = hT_d.name
            names["cfin"] = cT_d.name

            # ---- persistent SBUF tiles ----
            pp = stack.enter_context(tc.tile_pool(name="persist", bufs=1))
            ident128 = pp.tile([128, 128], fp32, tag="ident128")
            identBL = pp.tile([BL, BL], fp32, tag="identBL")
            ones1 = pp.tile([1, 128], fp32, tag="ones1")
            biasS = pp.tile([1, H4], fp32, tag="biasS")
            # transposed weights, one tile per contraction chunk: [128, 4H]
            WxT = []
            WhT = []
            for k in range(KC):
                WxT.append(pp.tile([128, H4], fp32, tag=f"wxT{k}", name=f"wxT{k}"))
                WhT.append(pp.tile([128, H4], fp32, tag=f"whT{k}", name=f"whT{k}"))
            h_sb = pp.tile([BL, H], fp32, tag="h_sb")
            c_sb = pp.tile([BL, H], fp32, tag="c_sb")
            hT_sb = pp.tile([128, KC * BL], fp32, tag="hT_sb")

            make_identity(nc, ident128[:])
            make_identity(nc, identBL[:])
            nc.gpsimd.memset(ones1[:], 1.0)
            nc.gpsimd.memset(h_sb[:], 0.0)
            nc.gpsimd.memset(c_sb[:], 0.0)
            nc.gpsimd.memset(hT_sb[:], 0.0)
            for gi, g in enumerate(GATES):
                nc.sync.dma_start(out=biasS[:, ts(gi, H)], in_=b_x[g][None, :])

            # ---- transpose weights: W[h,i] -> WT[i, g*H + h] ----
            with (
                tc.tile_pool(name="wload", bufs=4) as wl,
                tc.tile_pool(name="wps", bufs=4, space="PSUM") as wp,
            ):
                for (wsrc, wdst) in ((w_x, WxT), (w_h, WhT)):
                    for gi, g in enumerate(GATES):
                        for hc in range(H // 128):
                            for ic in range(KC):
                                wt = wl.tile([128, 128], fp32, tag="wt")
                                nc.sync.dma_start(
                                    out=wt[:],
                                    in_=wsrc[g][ts(hc, 128), ts(ic, 128)],
                                )
                                ps = wp.tile([128, 128], fp32, tag="wtp")
                                nc.tensor.transpose(ps[:], wt[:], ident128[:])
                                nc.vector.tensor_copy(
                                    out=wdst[ic][:, ds(gi * H + hc * 128, 128)],
                                    in_=ps[:],
                                )

            # ---- phase A: xz = x @ WxT + bias ----
            xf = x_d.flatten_outer_dims()        # [t*BL, I]
            xzf = xz_d.rearrange("n c b h -> (n c b) h")
            MT = (t_total * BL) // 128
            with (
                tc.tile_pool(name="xa", bufs=3) as xa,
                tc.tile_pool(name="xtp", bufs=4, space="PSUM") as xtp,
                tc.tile_pool(name="xts", bufs=8) as xts,
                tc.tile_pool(name="zps", bufs=4, space="PSUM") as zps,
                tc.tile_pool(name="aout", bufs=3) as aout,
            ):
                for m in range(MT):
                    xt = xa.tile([128, I], fp32, tag="xt")
                    nc.sync.dma_start(out=xt[:], in_=xf[ts(m, 128), :])
                    xT = []
                    for k in range(KC):
                        tp = xtp.tile([128, 128], fp32, tag="xtp")
                        nc.tensor.transpose(tp[:], xt[:, ts(k, 128)], ident128[:])
                        xs = xts.tile([128, 128], fp32, tag="xts")
                        nc.vector.tensor_copy(out=xs[:], in_=tp[:])
                        xT.append(xs)
                    ot = aout.tile([128, H4], fp32, tag="aout")
                    for n in range(H4 // 512):
                        zp = zps.tile([128, 512], fp32, tag="zp")
                        nc.tensor.matmul(
                            zp[:], lhsT=ones1[:], rhs=biasS[:, ts(n, 512)],
                            start=True, stop=False,
                        )
                        for k in range(KC):
                            nc.tensor.matmul(
                                zp[:], lhsT=xT[k][:], rhs=WxT[k][:, ts(n, 512)],
                                start=False, stop=(k == KC - 1),
                            )
                        nc.vector.tensor_copy(out=ot[:, ts(n, 512)], in_=zp[:])
                    nc.sync.dma_start(out=xzf[ts(m, 128), :], in_=ot[:])

            # ---- phase B: recurrence ----
            with (
                tc.tile_pool(name="xzp", bufs=3) as xzp,
                tc.tile_pool(name="zg", bufs=6, space="PSUM") as zg,
                tc.tile_pool(name="tps", bufs=2, space="PSUM") as tps,
                tc.tile_pool(name="gp", bufs=2) as gp,
            ):
                import concourse.mybir as _mb

                with tc.For_i(0, nch, 1, hint_engines=(_mb.EngineType.PE,)) as icnk:
                    for s in range(ch):
                        xzt = xzp.tile([BL, H4], fp32, tag="xzt")
                        nc.sync.dma_start(out=xzt[:], in_=xz_d[ds(icnk, 1)][0, s])
                        zpg = []
                        for gi in range(4):
                            zp = zg.tile([BL, 512], fp32, tag="zg")
                            nc.tensor.matmul(
                                zp[:], lhsT=identBL[:], rhs=xzt[:, ts(gi, 512)],
                                start=True, stop=False,
                            )
                            zpg.append(zp)
                        for k in range(KC):
                            for gi in range(4):
                                nc.tensor.matmul(
                                    zpg[gi][:],
                                    lhsT=hT_sb[:, ts(k, BL)],
                                    rhs=WhT[k][:, ts(gi, 512)],
                                    start=False, stop=(k == KC - 1),
                                )
                        i_t = gp.tile([BL, H], fp32, tag="i_t")
                        f_t = gp.tile([BL, H], fp32, tag="f_t")
                        o_t = gp.tile([BL, H], fp32, tag="o_t")
                        g_t = gp.tile([BL, H], fp32, tag="g_t")
                        nc.scalar.activation(i_t[:], zpg[0][:], AF.Sigmoid)
                        nc.scalar.activation(f_t[:], zpg[1][:], AF.Sigmoid)
                        nc.scalar.activation(o_t[:], zpg[2][:], AF.Sigmoid)
                        nc.scalar.activation(g_t[:], zpg[3][:], AF.Tanh)
                        fc = gp.tile([BL, H], fp32, tag="fc")
                        ig = gp.tile([BL, H], fp32, tag="ig")
                        nc.vector.tensor_tensor(
                            out=fc[:], in0=f_t[:], in1=c_sb[:], op=OP.mult)
                        nc.vector.tensor_tensor(
                            out=ig[:], in0=i_t[:], in1=g_t[:], op=OP.mult)
                        nc.vector.tensor_tensor(
                            out=c_sb[:], in0=fc[:], in1=ig[:], op=OP.add)
                        th = gp.tile([BL, H], fp32, tag="th")
                        nc.scalar.activation(th[:], c_sb[:], AF.Tanh)
                        nc.vector.tensor_tensor(
                            out=h_sb[:], in0=o_t[:], in1=th[:], op=OP.mult)
                        nc.sync.dma_start(out=y_d[ds(icnk, 1)][0, s], in_=h_sb[:])
                        for k in range(KC):
                            tp = tps.tile([128, BL], fp32, tag="tp")
                            nc.tensor.transpose(
                                tp[:], h_sb[:, ts(k, 128)], identBL[:])
                            nc.vector.tensor_copy(
                                out=hT_sb[:, ts(k, BL)], in_=tp[:])

            nc.sync.dma_start(out=hT_d[:, :], in_=h_sb[:])
            nc.sync.dma_start(out=cT_d[:, :], in_=c_sb[:])

    nc.compile()
    return nc, names


_CACHE = {}


def _compiled(t_total=T, ch=CH):
    key = (t_total, ch)
    if key not in _CACHE:
        _CACHE[key] = build(t_total, ch)
    return _CACHE[key]


def _in_maps(names, inputs, t_total=T):
    x = np.asarray(inputs["x"], np.float32)[:t_total]
    maps = []
    for r in range(NCORES):
        m = {names["x"]: np.ascontiguousarray(x[:, r * BL:(r + 1) * BL, :])}
        for g in GATES:
            m[names[f"wx{g}"]] = np.ascontiguousarray(inputs[f"W_x{g}"], np.float32)
            m[names[f"wh{g}"]] = np.ascontiguousarray(inputs[f"W_h{g}"], np.float32)
            m[names[f"bx{g}"]] = np.ascontiguousarray(inputs[f"b_x{g}"], np.float32)
        maps.append(m)
    return maps


def kernel(**inputs):
    from concourse import bass_utils

    nc, names = _compiled()
    maps = _in_maps(names, inputs)
    res = bass_utils.run_bass_kernel_spmd(nc, maps, core_ids=list(range(NCORES)))
    y = np.empty((T, B, H), np.float32)
    hf = np.empty((B, H), np.float32)
    cf = np.empty((B, H), np.float32)
    for r in range(NCORES):
        out = res.results[r]
        y[:, r * BL:(r + 1) * BL, :] = out[names["y"]].reshape(T, BL, H)
        hf[r * BL:(r + 1) * BL] = out[names["hfin"]]
        cf[r * BL:(r + 1) * BL] = out[names["cfin"]]
    return y, hf, cf


# revision 7
# speedup vs baseline: 1.2716x; 1.2716x over previous
"""LSTM layer kernel for trn2: T=2048, B=32, I=H=512, fp32.

Strategy: data-parallel over batch (B=32 -> 4 per core, 8 cores).
Per core, one bass program with two phases:
  Phase A: input projections xz[t,b,:] = concat_g(x[t,b] @ W_xg.T + b_g)
           as dense GEMMs (weights pre-transposed on chip via PE transpose),
           stored to a DRAM scratch [T, BL, 4H].
  Phase B: the sequential recurrence. State h kept both flat [BL,H] and
           transposed/packed hT [128, 4*BL] so the recurrent matmuls can use
           h.T chunks as the stationary operand:
             z_g[BL,512] = xz_g + sum_k hT[:,k].T @ WhT[k][:, g*512:+512]
           The xz term is injected into PSUM with an identity matmul.
           Gates via ScalarE sigmoid/tanh, c/h updates via VectorE,
           h -> hT via 4 small PE transposes per step.
"""

import numpy as np

T, B, I, H = 2048, 32, 512, 512
NCORES = 8
BL = B // NCORES            # 4 sequences per core
H4 = 4 * H                  # stacked gates i,f,o,c
CH = 16                     # steps per dynamic-loop chunk

GATES = ("i", "f", "o", "c")


def build(t_total=T, ch=CH):
    import concourse.bass as bass
    import concourse.mybir as mybir
    import concourse.tile as tile
    from concourse import bacc
    from concourse.bass import ds, ts
    from concourse.masks import make_identity

    fp32 = mybir.dt.float32
    AF = mybir.ActivationFunctionType
    OP = mybir.AluOpType
    nch = t_total // ch
    KC = I // 128            # 4 contraction chunks

    from contextlib import ExitStack

    nc = bacc.Bacc(None, target_bir_lowering=False)
    names = {}
    with tile.TileContext(nc) as tc:
        with ExitStack() as stack:
            dram = stack.enter_context(
                tc.tile_pool(name="dram", bufs=1, space="DRAM"))
            x_d = dram.tile([t_total, BL, I], fp32, kind="ExternalInput", name="x")
            w_x = {}
            w_h = {}
            b_x = {}
            for g in GATES:
                w_x[g] = dram.tile([H, I], fp32, kind="ExternalInput", name=f"wx{g}")
                w_h[g] = dram.tile([H, H], fp32, kind="ExternalInput", name=f"wh{g}")
                b_x[g] = dram.tile([H], fp32, kind="ExternalInput", name=f"bx{g}")
            y_d = dram.tile([nch, ch, BL, H], fp32, kind="ExternalOutput", name="y")
            hT_d = dram.tile([BL, H], fp32, kind="ExternalOutput", name="hfin")
            cT_d = dram.tile([BL, H], fp32, kind="ExternalOutput", name="cfin")
            xz_d = dram.tile([nch, ch, BL, H4], fp32, kind="Internal", name="xz")

            names["x"] = x_d.name
            for g in GATES:
                names[f"wx{g}"] = w_x[g].name
                names[f"wh{g}"] = w_h[g].name
                names[f"bx{g}"] = b_x[g].name
            names["y"] = y_d.name
            names["hfin"] = hT_d.name
            names["cfin"] = cT_d.name

            # ---- persistent SBUF tiles ----
            pp = stack.enter_context(tc.tile_pool(name="persist", bufs=1))
            ident128 = pp.tile([128, 128], fp32, tag="ident128")
            identBL = pp.tile([BL, BL], fp32, tag="identBL")
            ones1 = pp.tile([1, 128], fp32, tag="ones1")
            biasS = pp.tile([1, H4], fp32, tag="biasS")
            # transposed weights, one tile per contraction chunk: [128, 4H]
            WxT = []
            WhT = []
            for k in range(KC):
                WxT.append(pp.tile([128, H4], fp32, tag=f"wxT{k}", name=f"wxT{k}"))
                WhT.append(pp.tile([128, H4], fp32, tag=f"whT{k}", name=f"whT{k}"))
            # state double-buffered across even/odd steps to kill WAR stalls
            h_a = [pp.tile([BL, H], fp32, tag="h_a0", name="h_a0"),
                   pp.tile([BL, H], fp32, tag="h_a1", name="h_a1")]
            hT_a = [pp.tile([128, KC * BL], fp32, tag="hT_a0", name="hT_a0"),
                    pp.tile([128, KC * BL], fp32, tag="hT_a1", name="hT_a1")]
            c_sb = pp.tile([BL, H], fp32, tag="c_sb")

            make_identity(nc, ident128[:])
            make_identity(nc, identBL[:])
            nc.gpsimd.memset(ones1[:], 1.0)
            nc.gpsimd.memset(h_a[0][:], 0.0)
            nc.gpsimd.memset(h_a[1][:], 0.0)
            nc.gpsimd.memset(c_sb[:], 0.0)
            nc.gpsimd.memset(hT_a[0][:], 0.0)
            nc.gpsimd.memset(hT_a[1][:], 0.0)
            for gi, g in enumerate(GATES):
                nc.sync.dma_start(out=biasS[:, ts(gi, H)], in_=b_x[g][None, :])

            # ---- transpose weights: W[h,i] -> WT[i, g*H + h] ----
            with (
                tc.tile_pool(name="wload", bufs=4) as wl,
                tc.tile_pool(name="wps", bufs=4, space="PSUM") as wp,
            ):
                for (wsrc, wdst) in ((w_x, WxT), (w_h, WhT)):
                    for gi, g in enumerate(GATES):
                        for hc in range(H // 128):
                            for ic in range(KC):
                                wt = wl.tile([128, 128], fp32, tag="wt")
                                nc.sync.dma_start(
                                    out=wt[:],
                                    in_=wsrc[g][ts(hc, 128), ts(ic, 128)],
                                )
                                ps = wp.tile([128, 128], fp32, tag="wtp")
                                nc.tensor.transpose(ps[:], wt[:], ident128[:])
                                nc.vector.tensor_copy(
                                    out=wdst[ic][:, ds(gi * H + hc * 128, 128)],
                                    in_=ps[:],
                                )

            # ---- phase A: xz = x @ WxT + bias ----
            xf = x_d.flatten_outer_dims()        # [t*BL, I]
            xzf = xz_d.rearrange("n c b h -> (n c b) h")
            MT = (t_total * BL) // 128
            with (
                tc.tile_pool(name="xa", bufs=3) as xa,
                tc.tile_pool(name="xtp", bufs=4, space="PSUM") as xtp,
                tc.tile_pool(name="xts", bufs=8) as xts,
                tc.tile_pool(name="zps", bufs=4, space="PSUM") as zps,
                tc.tile_pool(name="aout", bufs=3) as aout,
            ):
                for m in range(MT):
                    xt = xa.tile([128, I], fp32, tag="xt")
                    nc.sync.dma_start(out=xt[:], in_=xf[ts(m, 128), :])
                    xT = []
                    for k in range(KC):
                        tp = xtp.tile([128, 128], fp32, tag="xtp")
                        nc.tensor.transpose(tp[:], xt[:, ts(k, 128)], ident128[:])
                        xs = xts.tile([128, 128], fp32, tag="xts")
                        nc.vector.tensor_copy(out=xs[:], in_=tp[:])
                        xT.append(xs)
                    ot = aout.tile([128, H4], fp32, tag="aout")
                    for n in range(H4 // 512):
                        zp = zps.tile([128, 512], fp32, tag="zp")
                        nc.tensor.matmul(
                            zp[:], lhsT=ones1[:], rhs=biasS[:, ts(n, 512)],
                            start=True, stop=False,
                        )
                        for k in range(KC):
                            nc.tensor.matmul(
                                zp[:], lhsT=xT[k][:], rhs=WxT[k][:, ts(n, 512)],
                                start=False, stop=(k == KC - 1),
                            )
                        nc.vector.tensor_copy(out=ot[:, ts(n, 512)], in_=zp[:])
                    nc.sync.dma_start(out=xzf[ts(m, 128), :], in_=ot[:])

            # ---- phase B: recurrence ----
            assert ch % 2 == 0
            with (
                tc.tile_pool(name="xzp", bufs=6) as xzp,
                tc.tile_pool(name="zg", bufs=6, space="PSUM") as zg,
                tc.tile_pool(name="tps", bufs=2, space="PSUM") as tps,
                tc.tile_pool(name="gp", bufs=3) as gp,
            ):
                import concourse.mybir as _mb

                # gate order: f first (frees fc mult early), o last before h
                GORD = (1, 0, 3, 2)  # indices into stacked [i,f,o,c]: f,i,c,o

                with tc.For_i(0, nch, 1, hint_engines=(_mb.EngineType.PE,)) as icnk:
                    for s in range(ch):
                        pr, pw = (s + 1) % 2, s % 2
                        hT_r = hT_a[pr]
                        h_w = h_a[pw]
                        xzt = xzp.tile([BL, H4], fp32, tag="xzt")
                        nc.sync.dma_start(out=xzt[:], in_=xz_d[ds(icnk, 1)][0, s])
                        zpg = {}
                        gact = {}
                        for gi in GORD:
                            zp = zg.tile([BL, 512], fp32, tag="zg")
                            nc.tensor.matmul(
                                zp[:], lhsT=identBL[:], rhs=xzt[:, ts(gi, 512)],
                                start=True, stop=False,
                            )
                            for k in range(KC):
                                nc.tensor.matmul(
                                    zp[:],
                                    lhsT=hT_r[:, ts(k, BL)],
                                    rhs=WhT[k][:, ts(gi, 512)],
                                    start=False, stop=(k == KC - 1),
                                )
                            zpg[gi] = zp
                            a_t = gp.tile([BL, H], fp32, tag=f"a{gi}",
                                          name=f"a{gi}")
                            nc.scalar.activation(
                                a_t[:], zp[:],
                                AF.Tanh if gi == 3 else AF.Sigmoid)
                            gact[gi] = a_t
                            if gi == 1:      # f done -> f*c
                                fc = gp.tile([BL, H], fp32, tag="fc")
                                nc.vector.tensor_tensor(
                                    out=fc[:], in0=a_t[:], in1=c_sb[:],
                                    op=OP.mult)
                            elif gi == 3:    # g done (i already done) -> i*g
                                ig = gp.tile([BL, H], fp32, tag="ig")
                                nc.vector.tensor_tensor(
                                    out=ig[:], in0=gact[0][:], in1=a_t[:],
                                    op=OP.mult)
                                nc.vector.tensor_tensor(
                                    out=c_sb[:], in0=fc[:], in1=ig[:],
                                    op=OP.add)
                                th = gp.tile([BL, H], fp32, tag="th")
                                nc.scalar.activation(th[:], c_sb[:], AF.Tanh)
                        nc.vector.tensor_tensor(
                            out=h_w[:], in0=gact[2][:], in1=th[:], op=OP.mult)
                        nc.sync.dma_start(out=y_d[ds(icnk, 1)][0, s], in_=h_w[:])
                        for k in range(KC):
                            tp = tps.tile([128, BL], fp32, tag="tp")
                            nc.tensor.transpose(
                                tp[:], h_w[:, ts(k, 128)], identBL[:])
                            nc.vector.tensor_copy(
                                out=hT_a[pw][:, ts(k, BL)], in_=tp[:])

            nc.sync.dma_start(out=hT_d[:, :], in_=h_a[(ch - 1) % 2][:])
            nc.sync.dma_start(out=cT_d[:, :], in_=c_sb[:])

    nc.compile()
    return nc, names


_CACHE = {}


def _compiled(t_total=T, ch=CH):
    key = (t_total, ch)
    if key not in _CACHE:
        _CACHE[key] = build(t_total, ch)
    return _CACHE[key]


def _in_maps(names, inputs, t_total=T):
    x = np.asarray(inputs["x"], np.float32)[:t_total]
    maps = []
    for r in range(NCORES):
        m = {names["x"]: np.ascontiguousarray(x[:, r * BL:(r + 1) * BL, :])}
        for g in GATES:
            m[names[f"wx{g}"]] = np.ascontiguousarray(inputs[f"W_x{g}"], np.float32)
            m[names[f"wh{g}"]] = np.ascontiguousarray(inputs[f"W_h{g}"], np.float32)
            m[names[f"bx{g}"]] = np.ascontiguousarray(inputs[f"b_x{g}"], np.float32)
        maps.append(m)
    return maps


def kernel(**inputs):
    from concourse import bass_utils

    nc, names = _compiled()
    maps = _in_maps(names, inputs)
    res = bass_utils.run_bass_kernel_spmd(nc, maps, core_ids=list(range(NCORES)))
    y = np.empty((T, B, H), np.float32)
    hf = np.empty((B, H), np.float32)
    cf = np.empty((B, H), np.float32)
    for r in range(NCORES):
        out = res.results[r]
        y[:, r * BL:(r + 1) * BL, :] = out[names["y"]].reshape(T, BL, H)
        hf[r * BL:(r + 1) * BL] = out[names["hfin"]]
        cf[r * BL:(r + 1) * BL] = out[names["cfin"]]
    return y, hf, cf
